# revision 35
# baseline (speedup 1.0000x reference)
"""Bahdanau attention, k-partitioned layout (v5). See kernel.py docstring.

Per core: batch b = c//2, queries [(c%2)*256, +256). Layout puts k (keys)
on partitions and h on the innermost free axis, so:
  - the h-reduction is a DVE free-axis tensor_reduce (no gpsimd big pass),
  - scores emerge as scT [k-part, q] = ready-made context-matmul lhsT
    (no transposes, no PSUM compaction),
  - softmax denominators are a tiny gpsimd all-reduce over k partitions.
Q is replicated across partitions once per q-tile (fold-DMA + gpsimd
partition_broadcast). All big tiles are persistent (no pool churn) and the
elementwise chain runs in place: this environment charges ~40us per
small/pool-cycled instruction but near-zero overhead for big in-place ones.
"""

import numpy as np

B, S, H = 4, 512, 256
NCORES = 8
QPC = (B * S) // NCORES  # 256
HP = 128
KC = S // 128            # 4 k-blocks
NQT = QPC // 128         # 2 q-tiles

_CACHE = {}


def _build(reps=1, skip=()):
    import concourse.bass as bass
    import concourse.tile as tile
    import concourse.mybir as mybir
    from concourse import bacc, bass_isa
    from contextlib import ExitStack

    f32 = mybir.dt.float32
    bf16 = mybir.dt.bfloat16
    AF = mybir.ActivationFunctionType
    ADD = mybir.AluOpType.add
    MUL = mybir.AluOpType.mult

    nc = bacc.Bacc("TRN2", target_bir_lowering=False, debug=False)

    hidT_u = nc.declare_dram_parameter("hidT_u", [HP, 2, S], f32, isOutput=False)
    hqT_u = nc.declare_dram_parameter("hqT_u", [HP, 2, QPC], f32, isOutput=False)
    hidb_u = nc.declare_dram_parameter("hidb_u", [HP, KC, H], f32, isOutput=False)
    WTp_u = nc.declare_dram_parameter("WTp_u", [HP, 2, H], f32, isOutput=False)
    UTp_u = nc.declare_dram_parameter("UTp_u", [HP, 2, H], f32, isOutput=False)
    vrep_u = nc.declare_dram_parameter("vrep_u", [HP, H], bf16, isOutput=False)
    out = nc.declare_dram_parameter("out", [QPC, H], f32, isOutput=True)
    dbg = None
    if "dbg" in skip:
        dbg = nc.declare_dram_parameter("dbg", [QPC, S], f32, isOutput=True)

    with tile.TileContext(nc) as tc, ExitStack() as ctx:
        sg = ctx.enter_context(tc.tile_pool(name="sg", bufs=1))
        psA = ctx.enter_context(tc.tile_pool(name="psA", bufs=1, space="PSUM"))

        hidT = sg.tile([HP, 2, S], f32, tag="hidT")
        nc.sync.dma_start(out=hidT, in_=hidT_u[:])
        hqT = sg.tile([HP, 2, QPC], f32, tag="hqT")
        nc.sync.dma_start(out=hqT, in_=hqT_u[:])
        hidb = sg.tile([HP, KC, H], f32, tag="hidb")
        nc.sync.dma_start(out=hidb, in_=hidb_u[:])
        WTp = sg.tile([HP, 2, H], f32, tag="WTp")
        nc.sync.dma_start(out=WTp, in_=WTp_u[:])
        UTp = sg.tile([HP, 2, H], f32, tag="UTp")
        nc.sync.dma_start(out=UTp, in_=UTp_u[:])
        vrep = sg.tile([HP, H], bf16, tag="vrep")
        nc.sync.dma_start(out=vrep, in_=vrep_u[:])

        # persistent work tiles
        X = sg.tile([HP, 128, H], bf16, tag="X")           # 64KB/part
        Qrep = sg.tile([HP, 128, H], bf16, tag="Qrep")     # 64KB/part
        Qs = sg.tile([HP, NQT, H], bf16, tag="Qs")
        scT = sg.tile([HP, NQT, KC, 128], f32, tag="scT")
        wtsT = sg.tile([HP, NQT, KC, 128], f32, tag="wtsT")
        ksum = sg.tile([HP, NQT, KC * 128], f32, tag="ksum")
        qsum = sg.tile([HP, NQT, 128], f32, tag="qsum")
        qsumT = sg.tile([HP, NQT], f32, tag="qsumT")
        rinv = sg.tile([HP, NQT], f32, tag="rinv")
        octx = sg.tile([HP, NQT, H], f32, tag="octx")

        KpP = psA.tile([HP, KC, S], f32, tag="KpP")    # 4 banks, 256 used/blk
        QnP = psA.tile([HP, NQT, S], f32, tag="QnP")   # 2 banks, 256 used/blk
        pctx = psA.tile([HP, S], f32, tag="pctx")      # 1 bank, 256 used

        for rep in range(reps):
            # Q projection first: its evac + the qt0 Qrep fold/broadcast
            # then overlap the K projection matmuls still on the PE queue.
            for qt in range(NQT):
                for hc in range(2):
                    nc.tensor.matmul(
                        QnP[:, qt, 0:H],
                        lhsT=hqT[:, hc, qt * 128 : (qt + 1) * 128],
                        rhs=WTp[:, hc, :], start=(hc == 0), stop=(hc == 1))
            nc.scalar.copy(Qs, QnP[:, :, 0:H])
            # K projection: Krow[k, h_out] per k-block (k on partitions)
            for kb in range(KC):
                for hc in range(2):
                    nc.tensor.matmul(
                        KpP[:, kb, 0:H],
                        lhsT=hidT[:, hc, kb * 128 : (kb + 1) * 128],
                        rhs=UTp[:, hc, :], start=(hc == 0), stop=(hc == 1))

            for qt in range(NQT):
                # replicate Q[q,h] of this q-tile to all partitions
                nc.sync.dma_start(
                    out=Qrep[0:1, :, :],
                    in_=Qs[:, qt, :])
                nc.gpsimd.partition_broadcast(
                    Qrep.rearrange("p q h -> p (q h)"),
                    Qrep[0:1].rearrange("p q h -> p (q h)"))
                for kb in range(KC):
                    k_b = bass.AP(
                        tensor=KpP.tensor, offset=KpP.offset + kb * S,
                        ap=[KpP.ap[0], [0, 128], [1, H]])
                    nc.vector.tensor_tensor(out=X, in0=k_b, in1=Qrep, op=ADD)
                    nc.scalar.activation(X, X, AF.Tanh)
                    v_b = bass.AP(
                        tensor=vrep.tensor, offset=vrep.offset,
                        ap=[vrep.ap[0], [0, 128], [1, H]])
                    nc.vector.tensor_tensor(out=X, in0=X, in1=v_b, op=MUL)
                    sl = scT[:, qt, kb, :]
                    nc.vector.tensor_reduce(
                        bass.AP(tensor=sl.tensor, offset=sl.offset,
                                ap=[*sl.ap, [1, 1]]), X,
                        axis=mybir.AxisListType.X, op=ADD)
                # softmax pieces: exp, k-partition sums, fold k-blocks
                nc.scalar.activation(wtsT[:, qt], scT[:, qt], AF.Exp)
                nc.gpsimd.partition_all_reduce(
                    ksum[:, qt], wtsT[:, qt].rearrange("p a b -> p (a b)"),
                    channels=HP, reduce_op=bass_isa.ReduceOp.add)
                kv = bass.AP(
                    tensor=ksum.tensor, offset=ksum.offset + qt * (KC * 128),
                    ap=[ksum.ap[0], [1, 128], [128, KC]])
                qsl = qsum[:, qt]
                nc.vector.tensor_reduce(
                    bass.AP(tensor=qsl.tensor, offset=qsl.offset,
                            ap=[*qsl.ap, [1, 1]]), kv,
                    axis=mybir.AxisListType.X, op=ADD)
                nc.sync.dma_start(
                    out=qsumT[:, qt : qt + 1], in_=qsum[0:1, qt, :])
                # context
                for kb in range(KC):
                    nc.tensor.matmul(
                        pctx[:, 0:H], lhsT=wtsT[:, qt, kb, :],
                        rhs=hidb[:, kb, :], start=(kb == 0), stop=(kb == KC - 1))
                nc.vector.reciprocal(rinv[:, qt : qt + 1], qsumT[:, qt : qt + 1])
                nc.vector.tensor_scalar_mul(
                    octx[:, qt, :], pctx[:, 0:H], rinv[:, qt : qt + 1])
                nc.sync.dma_start(
                    out=out[qt * 128 : (qt + 1) * 128, :], in_=octx[:, qt, :])
                if dbg is not None and rep == 0:
                    dsc = sg.tile([128, S], f32, tag=f"dsc{qt}")
                    nc.vector.tensor_copy(
                        dsc, scT[:, qt].rearrange("p a b -> p (a b)"))
                    nc.sync.dma_start(
                        out=dbg[qt * 128 : (qt + 1) * 128, :], in_=dsc)

    nc.compile()
    return nc


def _get(reps=1, skip=()):
    key = (reps, tuple(skip))
    if key not in _CACHE:
        _CACHE[key] = _build(reps, skip)
    return _CACHE[key]


def _to_bf16(a):
    import ml_dtypes
    return np.ascontiguousarray(a).astype(ml_dtypes.bfloat16)


def _in_maps(hs, W, U, v):
    hs = np.asarray(hs, np.float32)
    WTh = np.asarray(W, np.float32).T
    UTh = np.asarray(U, np.float32).T
    WTp = np.ascontiguousarray(WTh.reshape(2, HP, H).transpose(1, 0, 2))
    UTp = np.ascontiguousarray(UTh.reshape(2, HP, H).transpose(1, 0, 2))
    vrep = np.tile(np.asarray(v, np.float32)[None, :], (HP, 1))
    maps = []
    for c in range(NCORES):
        b, qh = divmod(c, 2)
        hT = hs[b].T
        hidT = np.ascontiguousarray(hT.reshape(2, HP, S).transpose(1, 0, 2))
        hqT = np.ascontiguousarray(
            hT[:, qh * QPC : (qh + 1) * QPC].reshape(2, HP, QPC).transpose(1, 0, 2))
        hidb = hs[b].reshape(KC, HP, H).transpose(1, 0, 2)
        maps.append({
            "hidT_u": hidT,
            "hqT_u": hqT,
            "hidb_u": hidb.astype(np.float32),
            "WTp_u": WTp,
            "UTp_u": UTp,
            "vrep_u": _to_bf16(vrep),
        })
    return maps


def run(hidden_states, W, U, v, reps=1, skip=()):
    from concourse.bass_utils import run_bass_kernel_spmd

    nc = _get(reps, skip)
    res = run_bass_kernel_spmd(
        nc, _in_maps(hidden_states, W, U, v), core_ids=list(range(NCORES)))
    ctxout = np.empty((B, S, H), np.float32)
    for c in range(NCORES):
        b, qh = divmod(c, 2)
        ctxout[b, qh * QPC : (qh + 1) * QPC] = res.results[c]["out"]
    return ctxout


def kernel(**inputs):
    return run(inputs["hidden_states"], inputs["W"], inputs["U"], inputs["v"])
